# revision 14
# baseline (speedup 1.0000x reference)
"""Char-level BiLSTM embedder on 8 NeuronCores (Trainium2, Bass/Tile).

Computation: x[B=32,T=128,L=16] char ids -> embed[E=512] -> fwd+bwd LSTM(H=256)
over the L=16 chars of each of the N=B*T=4096 independent words -> final hidden
states concatenated -> y[B,T,2H=512].

v2 strategy (vs v1: 24 MMs/step-dir + all 5 activations on ACT):
  - Data parallel over N: 512 words per core; everything feature-major.
  - Gates f,o,g get their input projection via one-hot matmuls on PE
    (fused LUT = embed@w_ih.T+b), accumulated with the recurrent h matmuls
    in PSUM.  Gate i's input projection is precomputed on HOST for all
    steps, DMA'd, and fused into a custom DVE op (ADDSIG2) -> 2 fewer MMs
    per step-dir on the PE bottleneck (22 instead of 24).
  - Work split per step-dir:
      ACT : sigmoid(f), sigmoid(o), tanh(g)
      DVE : s2i = 1+2p(psum_i+xw_i) = 2*sigmoid(i)   (custom ADDSIG2)
            m2  = s2i * tanh_g                        (stock bf16 2x)
            c2  = m1 + m2          (c2 = 2c)          (stock bf16 2x)
            h   = p(c2)*sig_o  = tanh(c)*sig_o        (custom TANHMUL)
      GPSIMD: m1 = sig_f * c2_prev                    (off critical path)
    where p(x) ~= tanh(x/2), a degree-5 odd minimax poly (err < 7e-6 on
    the empirical preact/cell range |x| < 0.75).
  - PSUM as four 2-bank tiles (f/o/g/i) x bufs=4 = all 8 banks: fine-
    grained recycling so the other direction's matmuls never wait on a
    big activation drain.
  - The 2x cell-state trick keeps every stored tensor an exact LSTM
    quantity (c2 = 2c), so weights/LUT/outputs need no rescaling.
"""

import sys

sys.path.insert(0, "/opt/trn_rl_repo")

import numpy as np
import concourse.bass as bass
import concourse.bacc as bacc
import concourse.mybir as mybir
import concourse.tile as tile
from concourse.bass_utils import run_bass_kernel_spmd

# ---------------------------------------------------------------- custom DVE
import concourse.dve_ops as dops
from concourse.dve_spec import Spec, Src0, Src1, C0, C1, C2, One, lower, spec_leaves
from concourse.dve_uop import DveOpSpec

# tanh(x/2) ~= x*(P0 + u*(P1 + u*P2)), u = x^2, minimax on [-0.9, 0.9]
P0, P1, P2 = 0.49998416, -0.04148776, 0.00366485


def _register_dve_op(name, spec):
    if name in dops._SUB_OPCODE_FOR_NAME:
        return next(op for op in dops.OPS if op.name == name)
    row = dops._CUSTOM_DVE_ROW_BASE + len(dops.OPS)
    assert row < 0x20, "custom-DVE row field overflow"
    dops._SUB_OPCODE_FOR_NAME[name] = row
    has_src1 = Src1 in spec_leaves(spec)
    shas = {
        ver: DveOpSpec(
            name=name, opcode=row, uops=lower(spec, ver=ver), rd1_en=has_src1
        ).sha(ver)
        for ver in ("v3", "v4")
    }
    op = dops.DveOp(name, spec, subdim=False, uops_sha=shas)
    dops.OPS.append(op)
    dops.CUSTOM_DVE_SPECS[name] = spec
    return op


def _mk_specs():
    # ADDSIG2: out = 1 + p(Src0+Src1) = 2*sigmoid(Src0+Src1)
    s = Src0 + Src1
    u = s * s
    addsig2 = Spec(
        body=One + s * (C0 + u * (C1 + u * C2)),
        reference=lambda in0, in1, s0, s1, imm2: (
            lambda t: (1.0 + t * (s0 + t * t * (s1 + t * t * imm2))).astype(np.float32)
        )(np.asarray(in0, np.float32) + np.asarray(in1, np.float32)),
    )
    # TANHMUL: out = p(Src0) * Src1 = tanh(Src0/2) * Src1
    v = Src0
    w = v * v
    tanhmul = Spec(
        body=v * (C0 + w * (C1 + w * C2)) * Src1,
        reference=lambda in0, in1, s0, s1, imm2: (
            np.asarray(in0, np.float32)
            * (s0 + np.asarray(in0, np.float32) ** 2
               * (s1 + np.asarray(in0, np.float32) ** 2 * imm2))
            * np.asarray(in1, np.float32)
        ).astype(np.float32),
    )
    return addsig2, tanhmul


_SPEC_ADDSIG2, _SPEC_TANHMUL = _mk_specs()
ADDSIG2 = _register_dve_op("ADDSIG2_CLBS", _SPEC_ADDSIG2)
TANHMUL = _register_dve_op("TANHMUL_CLBS", _SPEC_TANHMUL)

# ---------------------------------------------------------------- constants
B, T, L = 32, 128, 16
VOCAB, E, H = 128, 512, 256
N_CORES = 8
NW = (B * T) // N_CORES  # 512 words per core

F32 = mybir.dt.float32
DT = mybir.dt.bfloat16
F8 = mybir.dt.float8e4
AFT = mybir.ActivationFunctionType

# fp8(e4m3) recurrent path: whh + h stored fp8, the 16 bf16 h-matmuls per
# step-dir become 8 DoubleRow matmuls (K=256 in one pass).  numpy-simulated
# end-to-end rel err 1.36e-2 (vs 4.4e-3 bf16), tolerance 2e-2.
USE_FP8_H = True

# gate rows in torch order within w_ih/w_hh/b: i 0:256, f 256:512, g 512:768,
# o 768:1024.  lhsT chunk order used on-device: f0 f1 o0 o1 g0 g1 (+ i0 i1
# for the recurrent weights).
_FOG_COLS = np.concatenate(
    [np.arange(256, 512), np.arange(768, 1024), np.arange(512, 768)]
)
_HSLICE_COLS = np.concatenate(
    [np.arange(256, 512), np.arange(768, 1024), np.arange(512, 768),
     np.arange(0, 256)]
)

# gate -> index into the f,o,g,i slice layouts
_GSL = {"f": 0, "o": 1, "g": 2, "i": 3}
# h-matmul emission order: f first (sigmoid_f -> m1 is the longest chain),
# then g (tanh_g), i (ADDSIG2), o (sigmoid_o, needed last)
_H_ORDER = ("f", "g", "i", "o")
_OH_ORDER = ("f", "g", "o")


def build_nc():
    nc = bacc.Bacc()

    oh_d = nc.dram_tensor("oh", [L, VOCAB, NW], DT, kind="ExternalInput")
    # xwi[s = t*2 + dir] = gate-i input projection, [128, 2*NW] (k*NW + word)
    xwi_d = nc.dram_tensor("xwi", [2 * L, 128, 2 * NW], DT, kind="ExternalInput")
    fused_dd = {
        d: nc.dram_tensor(f"fused_{d}", [VOCAB, 6 * 128], DT, kind="ExternalInput")
        for d in "fb"
    }
    WDT = F8 if USE_FP8_H else DT
    whh_dd = {
        d: nc.dram_tensor(f"whh_{d}", [2, 128, 8 * 128], WDT, kind="ExternalInput")
        for d in "fb"
    }
    hout_d = nc.dram_tensor("hout", [128, 4 * NW], DT, kind="ExternalOutput")

    with tile.TileContext(nc) as tc:
        with (
            tc.tile_pool(name="const", bufs=1) as cpool,
            tc.tile_pool(name="work", bufs=2) as wpool,
            tc.tile_pool(name="state", bufs=2) as spool,
            tc.tile_pool(name="psum", bufs=4, space=bass.MemorySpace.PSUM) as ppool,
        ):
            # --- constants / inputs --------------------------------------
            # t=0 slabs first (small DMAs) so step 0 can start immediately.
            xwi_sb = cpool.tile([128, 2 * L * 2 * NW], DT, name="xwi_sb", tag="xwi")
            xwi3 = xwi_sb[:].rearrange("p (s n) -> p s n", s=2 * L)
            nc.sync.dma_start(xwi3[:, 0:2], xwi_d[0:2].rearrange("s p n -> p s n"))

            fused = {}
            oh_ends = {}
            for d, te in (("f", 0), ("b", L - 1)):
                fu = cpool.tile([128, 6 * 128], DT, name=f"fused_{d}_sb", tag=f"fu_{d}")
                nc.sync.dma_start(fu[:], fused_dd[d][:])
                fused[d] = fu
                ot = cpool.tile([128, NW], DT, name=f"oh_e{te}", tag=f"oh_e{te}")
                nc.sync.dma_start(ot[:], oh_d[te])
                oh_ends[te] = ot
            whh = {}
            for d in "fb":
                w = cpool.tile([128, 2 * 8 * 128], WDT, name=f"whh_{d}_sb", tag=f"whh_{d}")
                nc.sync.dma_start(
                    w[:].rearrange("p (k g) -> p k g", k=2),
                    whh_dd[d].rearrange("k p g -> p k g"),
                )
                if USE_FP8_H:
                    # 3D view [p, k=2, 8*128] for DoubleRow lhsT slicing
                    whh[d] = w[:].rearrange("p (k g) -> p k g", k=2)
                else:
                    whh[d] = [w[:, 0 : 8 * 128], w[:, 8 * 128 : 16 * 128]]
            oh_mid = {}
            for lo, hi in ((1, 8), (8, 15)):
                om = cpool.tile([128, 7 * NW], DT, name=f"oh_m{lo}", tag=f"oh_m{lo}")
                nc.sync.dma_start(
                    om[:].rearrange("p (t n) -> p t n", t=7),
                    oh_d[lo:hi].rearrange("t p n -> p t n"),
                )
                oh_mid[lo] = om
            # rest of xwi, per-step slabs so step t only waits on its own pair
            for s in range(2, 2 * L, 2):
                nc.sync.dma_start(
                    xwi3[:, s : s + 2],
                    xwi_d[s : s + 2].rearrange("s p n -> p s n"),
                )

            def oh_rhs(t):
                if t in oh_ends:
                    return oh_ends[t][:]
                lo = 1 if t < 8 else 8
                return oh_mid[lo][:, (t - lo) * NW : (t - lo + 1) * NW]

            def xwi_slice(d, t):
                s = 2 * t + (0 if d == "f" else 1)
                return xwi_sb[:, s * 2 * NW : (s + 1) * 2 * NW]

            out_sb = cpool.tile([128, 4 * NW], DT, name="out_sb", tag="out_sb")
            zero_sb = cpool.tile([128, 2 * NW], DT, name="zero_sb", tag="zero_sb")
            nc.vector.memset(zero_sb[:], 0.0)
            # GPSIMD warm-up: the first tensor op on Pool pays a ~6us IRAM
            # kernel load; pay it here (overlapped with input DMAs) instead of
            # on the t=1 cell-update chain.
            gp_warm = cpool.tile([128, 128], DT, name="gp_warm", tag="gp_warm")
            nc.gpsimd.tensor_mul(gp_warm[:], zero_sb[:, 0:128], zero_sb[:, 0:128])
            gp_warm8 = cpool.tile([128, 128], F8, name="gp_warm8", tag="gp_warm8")
            nc.gpsimd.tensor_copy(gp_warm8[:], gp_warm[:])

            # HAM warm-up: dummy matmuls while input DMAs are in flight.
            warm_src = wpool.tile([128, NW], DT, name="warm_src", tag="warm_src", bufs=1)
            nc.gpsimd.memset(warm_src[:], 0.0)
            warm_ps = ppool.tile([128, 2 * NW], F32, name="warm_ps", tag="ps")
            for wj in range(22):
                nc.tensor.matmul(
                    warm_ps[:, (wj % 8) * 128 : (wj % 8) * 128 + 128],
                    warm_src[:, 0:128],
                    warm_src[:, 0:128],
                    start=True,
                    stop=True,
                )

            c2_cur = {"f": None, "b": None}
            h_cur = {"f": None, "b": None}

            # --- per-step-dir emission helpers ----------------------------
            def emit_mms(d, t):
                tchar = t if d == "f" else L - 1 - t
                rhs_oh = oh_rhs(tchar)
                h_prev = h_cur[d]
                first = h_prev is None
                ps = {}
                # one-hot LUT matmuls (gate i has none; fused col order f,o,g)
                for g in _OH_ORDER:
                    ps[g] = ppool.tile([128, 2 * NW], F32, name=f"p_{g}", tag="ps")
                    oc = 2 * _GSL[g]
                    for j in range(2):
                        nc.tensor.matmul(
                            ps[g][:, j * NW : (j + 1) * NW],
                            fused[d][:, (oc + j) * 128 : (oc + j + 1) * 128],
                            rhs_oh,
                            start=True,
                            stop=first,
                        )
                if not first:
                    ps["i"] = ppool.tile([128, 2 * NW], F32, name="p_i", tag="ps")
                    # recurrent matmuls; whh slice order f0 f1 o0 o1 g0 g1 i0 i1
                    if USE_FP8_H:
                        rhs_h = h_prev[:].rearrange("p (k n) -> p k n", k=2)
                        for g in _H_ORDER:
                            for j in range(2):
                                sl = 2 * _GSL[g] + j
                                nc.tensor.matmul(
                                    ps[g][:, j * NW : (j + 1) * NW],
                                    whh[d][:, :, sl * 128 : (sl + 1) * 128],
                                    rhs_h,
                                    start=(g == "i"),
                                    stop=True,
                                    perf_mode=mybir.MatmulPerfMode.DoubleRow,
                                )
                    else:
                        for g in _H_ORDER:
                            for j in range(2):
                                sl = 2 * _GSL[g] + j
                                for k in range(2):
                                    nc.tensor.matmul(
                                        ps[g][:, j * NW : (j + 1) * NW],
                                        whh[d][k][:, sl * 128 : (sl + 1) * 128],
                                        h_prev[:, k * NW : (k + 1) * NW],
                                        start=(g == "i" and k == 0),
                                        stop=(k == 1),
                                    )
                return ps

            def emit_act(d, ps):
                sf = wpool.tile([128, 2 * NW], DT, name="sf", tag=f"sf_{d}")
                nc.scalar.activation(sf[:], ps["f"][:], AFT.Sigmoid)
                so = wpool.tile([128, 2 * NW], DT, name="so", tag=f"so_{d}")
                nc.scalar.activation(so[:], ps["o"][:], AFT.Sigmoid)
                tg = wpool.tile([128, 2 * NW], DT, name="tg", tag=f"tg_{d}")
                nc.scalar.activation(tg[:], ps["g"][:], AFT.Tanh)
                return sf, so, tg

            def emit_cell(d, t, ps, sf, tg):
                # s2i = 2*sigmoid(i)
                s2i = wpool.tile([128, 2 * NW], DT, name="s2i", tag=f"s2i_{d}")
                if t == 0:
                    nc.vector._custom_dve(
                        ADDSIG2, out=s2i[:], in0=xwi_slice(d, t), in1=zero_sb[:],
                        s0=P0, s1=P1, imm2=P2,
                    )
                else:
                    nc.vector._custom_dve(
                        ADDSIG2, out=s2i[:], in0=ps["i"][:], in1=xwi_slice(d, t),
                        s0=P0, s1=P1, imm2=P2,
                    )
                # m2 = 2*sig_i*tanh_g
                m2 = wpool.tile([128, 2 * NW], DT, name="m2", tag=f"m2_{d}")
                nc.vector.tensor_mul(m2[:], s2i[:], tg[:])
                if t == 0:
                    c2_cur[d] = m2
                    return
                # m1 = sig_f * c2_prev  (GPSIMD: off the critical path)
                m1 = wpool.tile([128, 2 * NW], DT, name="m1", tag=f"m1_{d}")
                nc.gpsimd.tensor_mul(m1[:], sf[:], c2_cur[d][:])
                c2 = spool.tile([128, 2 * NW], DT, name=f"c2_{d}", tag=f"c2_{d}")
                nc.vector.tensor_add(c2[:], m1[:], m2[:])
                c2_cur[d] = c2

            def emit_h(d, t, so):
                # h = tanh(c) * sig_o = p(c2) * sig_o
                if t == L - 1:
                    off = 0 if d == "f" else 2 * NW
                    out_ap = out_sb[:, off : off + 2 * NW]
                    h = None
                else:
                    h = spool.tile([128, 2 * NW], DT, name=f"h_{d}", tag=f"h_{d}")
                    out_ap = h[:]
                    h_cur[d] = h
                nc.vector._custom_dve(
                    TANHMUL, out=out_ap, in0=c2_cur[d][:], in1=so[:],
                    s0=P0, s1=P1, imm2=P2,
                )
                if USE_FP8_H and h is not None:
                    # fp8 copy for the DoubleRow rhs (8-bit DVE writes run at
                    # half rate, so convert on GPSIMD instead)
                    h8 = spool.tile([128, 2 * NW], F8, name=f"h8_{d}", tag=f"h8_{d}")
                    nc.gpsimd.tensor_copy(h8[:], h[:])
                    h_cur[d] = h8

            # --- the recurrence, f/b interleaved --------------------------
            for t in range(L):
                ps_f = emit_mms("f", t)
                sf_f, so_f, tg_f = emit_act("f", ps_f)
                ps_b = emit_mms("b", t)
                emit_cell("f", t, ps_f, sf_f, tg_f)
                sf_b, so_b, tg_b = emit_act("b", ps_b)
                emit_h("f", t, so_f)
                emit_cell("b", t, ps_b, sf_b, tg_b)
                emit_h("b", t, so_b)

            nc.sync.dma_start(hout_d[:, 0 : 2 * NW], out_sb[:, 0 : 2 * NW])
            nc.sync.dma_start(hout_d[:, 2 * NW : 4 * NW], out_sb[:, 2 * NW : 4 * NW])

    nc.compile()
    return nc


_NC_CACHE = None


def _get_nc():
    global _NC_CACHE
    if _NC_CACHE is None:
        _NC_CACHE = build_nc()
    return _NC_CACHE


def _np_dt(dt):
    return mybir.dt.np(dt)


def prepare_in_maps(x, embed_table, w_ih_f, w_hh_f, b_ih_f, b_hh_f,
                    w_ih_b, w_hh_b, b_ih_b, b_hh_b):
    cdt = _np_dt(DT)
    ids = np.asarray(x).reshape(B * T, L).astype(np.int64)

    shared = {}
    fused_i = {}
    for d, w_ih, w_hh, b_ih, b_hh in (
        ("f", w_ih_f, w_hh_f, b_ih_f, b_hh_f),
        ("b", w_ih_b, w_hh_b, b_ih_b, b_hh_b),
    ):
        w_ih = np.asarray(w_ih, np.float32)
        w_hh = np.asarray(w_hh, np.float32)
        b = np.asarray(b_ih, np.float32) + np.asarray(b_hh, np.float32)
        fused_full = np.asarray(embed_table, np.float32) @ w_ih.T + b[None, :]
        shared[f"fused_{d}"] = np.ascontiguousarray(
            fused_full[:, _FOG_COLS].astype(cdt)
        )
        fused_i[d] = np.ascontiguousarray(fused_full[:, 0:256].astype(cdt))
        whh_t = w_hh.T[:, _HSLICE_COLS]  # [256, 8*128] cols = slice-major
        wdt = _np_dt(F8) if USE_FP8_H else cdt
        shared[f"whh_{d}"] = np.ascontiguousarray(
            whh_t.reshape(2, 128, 8 * 128).astype(wdt)
        )

    vrange = np.arange(VOCAB)
    in_maps = []
    for c in range(N_CORES):
        ids_c = ids[c * NW : (c + 1) * NW]  # [NW, L]
        oh = (ids_c.T[:, None, :] == vrange[None, :, None]).astype(cdt)  # [L,V,NW]
        # xwi[s=2t+dir]: [128, 2*NW] with col = k*NW + word
        xwi = np.empty((2 * L, 128, 2 * NW), cdt)
        for t in range(L):
            for di, d in enumerate("fb"):
                tchar = t if d == "f" else L - 1 - t
                g = fused_i[d][ids_c[:, tchar]]  # [NW, 256]
                xwi[2 * t + di] = (
                    g.T.reshape(2, 128, NW).transpose(1, 0, 2).reshape(128, 2 * NW)
                )
        m = dict(shared)
        m["oh"] = np.ascontiguousarray(oh)
        m["xwi"] = np.ascontiguousarray(xwi)
        in_maps.append(m)
    return in_maps


def assemble_output(results):
    ys = []
    for c in range(N_CORES):
        hout = results[c]["hout"].astype(np.float32)  # [128, 4*NW]
        hf = np.concatenate([hout[:, 0:NW], hout[:, NW : 2 * NW]], axis=0)  # [H,NW]
        hb = np.concatenate([hout[:, 2 * NW : 3 * NW], hout[:, 3 * NW : 4 * NW]], axis=0)
        ys.append(np.concatenate([hf.T, hb.T], axis=1))  # [NW, 2H]
    y = np.concatenate(ys, axis=0)  # [B*T, 2H]
    return y.reshape(B, T, 2 * H)


def run(in_maps, trace=False):
    nc = _get_nc()
    res = run_bass_kernel_spmd(nc, in_maps, core_ids=list(range(N_CORES)), trace=trace)
    return res


def kernel(**inputs) -> np.ndarray:
    in_maps = prepare_in_maps(**inputs)
    res = run(in_maps, trace=False)
    return assemble_output(res.results)


# revision 15
# speedup vs baseline: 1.6072x; 1.6072x over previous
"""Char-level BiLSTM embedder on 8 NeuronCores (Trainium2, Bass/Tile).

Computation: x[B=32,T=128,L=16] char ids -> embed[E=512] -> fwd+bwd LSTM(H=256)
over the L=16 chars of each of the N=B*T=4096 independent words -> final hidden
states concatenated -> y[B,T,2H=512].

v2 strategy (vs v1: 24 MMs/step-dir + all 5 activations on ACT):
  - Data parallel over N: 512 words per core; everything feature-major.
  - Gates f,o,g get their input projection via one-hot matmuls on PE
    (fused LUT = embed@w_ih.T+b), accumulated with the recurrent h matmuls
    in PSUM.  Gate i's input projection is precomputed on HOST for all
    steps, DMA'd, and fused into a custom DVE op (ADDSIG2) -> 2 fewer MMs
    per step-dir on the PE bottleneck (22 instead of 24).
  - Work split per step-dir:
      ACT : sigmoid(f), sigmoid(o), tanh(g)
      DVE : s2i = 1+2p(psum_i+xw_i) = 2*sigmoid(i)   (custom ADDSIG2)
            m2  = s2i * tanh_g                        (stock bf16 2x)
            c2  = m1 + m2          (c2 = 2c)          (stock bf16 2x)
            h   = p(c2)*sig_o  = tanh(c)*sig_o        (custom TANHMUL)
      GPSIMD: m1 = sig_f * c2_prev                    (off critical path)
    where p(x) ~= tanh(x/2), a degree-5 odd minimax poly (err < 7e-6 on
    the empirical preact/cell range |x| < 0.75).
  - PSUM as four 2-bank tiles (f/o/g/i) x bufs=4 = all 8 banks: fine-
    grained recycling so the other direction's matmuls never wait on a
    big activation drain.
  - The 2x cell-state trick keeps every stored tensor an exact LSTM
    quantity (c2 = 2c), so weights/LUT/outputs need no rescaling.
"""

import sys

sys.path.insert(0, "/opt/trn_rl_repo")

import numpy as np
import concourse.bass as bass
import concourse.bacc as bacc
import concourse.mybir as mybir
import concourse.tile as tile
from concourse.bass_utils import run_bass_kernel_spmd

# ---------------------------------------------------------------- custom DVE
import concourse.dve_ops as dops
from concourse.dve_spec import Spec, Src0, Src1, C0, C1, C2, One, lower, spec_leaves
from concourse.dve_uop import DveOpSpec

# tanh(x/2) ~= x*(P0 + u*(P1 + u*P2)), u = x^2, minimax on [-0.9, 0.9]
P0, P1, P2 = 0.49998416, -0.04148776, 0.00366485


def _register_dve_op(name, spec):
    if name in dops._SUB_OPCODE_FOR_NAME:
        return next(op for op in dops.OPS if op.name == name)
    row = dops._CUSTOM_DVE_ROW_BASE + len(dops.OPS)
    assert row < 0x20, "custom-DVE row field overflow"
    dops._SUB_OPCODE_FOR_NAME[name] = row
    has_src1 = Src1 in spec_leaves(spec)
    shas = {
        ver: DveOpSpec(
            name=name, opcode=row, uops=lower(spec, ver=ver), rd1_en=has_src1
        ).sha(ver)
        for ver in ("v3", "v4")
    }
    op = dops.DveOp(name, spec, subdim=False, uops_sha=shas)
    dops.OPS.append(op)
    dops.CUSTOM_DVE_SPECS[name] = spec
    return op


def _mk_specs():
    # ADDSIG2: out = 1 + p(Src0+Src1) = 2*sigmoid(Src0+Src1)
    s = Src0 + Src1
    u = s * s
    addsig2 = Spec(
        body=One + s * (C0 + u * (C1 + u * C2)),
        reference=lambda in0, in1, s0, s1, imm2: (
            lambda t: (1.0 + t * (s0 + t * t * (s1 + t * t * imm2))).astype(np.float32)
        )(np.asarray(in0, np.float32) + np.asarray(in1, np.float32)),
    )
    # TANHMUL: out = p(Src0) * Src1 = tanh(Src0/2) * Src1
    v = Src0
    w = v * v
    tanhmul = Spec(
        body=v * (C0 + w * (C1 + w * C2)) * Src1,
        reference=lambda in0, in1, s0, s1, imm2: (
            np.asarray(in0, np.float32)
            * (s0 + np.asarray(in0, np.float32) ** 2
               * (s1 + np.asarray(in0, np.float32) ** 2 * imm2))
            * np.asarray(in1, np.float32)
        ).astype(np.float32),
    )
    return addsig2, tanhmul


_SPEC_ADDSIG2, _SPEC_TANHMUL = _mk_specs()
ADDSIG2 = _register_dve_op("ADDSIG2_CLBS", _SPEC_ADDSIG2)
TANHMUL = _register_dve_op("TANHMUL_CLBS", _SPEC_TANHMUL)

# ---------------------------------------------------------------- constants
B, T, L = 32, 128, 16
VOCAB, E, H = 128, 512, 256
N_CORES = 8
NW = (B * T) // N_CORES  # 512 words per core

F32 = mybir.dt.float32
DT = mybir.dt.bfloat16
F8 = mybir.dt.float8e4
AFT = mybir.ActivationFunctionType

# fp8(e4m3) recurrent path: whh + h stored fp8, the 16 bf16 h-matmuls per
# step-dir become 8 DoubleRow matmuls (K=256 in one pass).  numpy-simulated
# end-to-end rel err 1.36e-2 (vs 4.4e-3 bf16), tolerance 2e-2.
USE_FP8_H = True

# gate rows in torch order within w_ih/w_hh/b: i 0:256, f 256:512, g 512:768,
# o 768:1024.  lhsT chunk order used on-device: f0 f1 o0 o1 g0 g1 (+ i0 i1
# for the recurrent weights).
_FOG_COLS = np.concatenate(
    [np.arange(256, 512), np.arange(768, 1024), np.arange(512, 768)]
)
_HSLICE_COLS = np.concatenate(
    [np.arange(256, 512), np.arange(768, 1024), np.arange(512, 768),
     np.arange(0, 256)]
)

# gate -> index into the f,o,g,i slice layouts
_GSL = {"f": 0, "o": 1, "g": 2, "i": 3}
# h-matmul emission order: f first (sigmoid_f -> m1 is the longest chain),
# then g (tanh_g), i (ADDSIG2), o (sigmoid_o, needed last)
_H_ORDER = ("f", "g", "i", "o")
_OH_ORDER = ("f", "g", "o")


def build_nc():
    nc = bacc.Bacc()

    oh_d = nc.dram_tensor("oh", [L, VOCAB, NW], DT, kind="ExternalInput")
    # xwi[s = t*2 + dir] = gate-i input projection, [128, 2*NW] (k*NW + word)
    xwi_d = nc.dram_tensor("xwi", [2 * L, 128, 2 * NW], DT, kind="ExternalInput")
    fused_dd = {
        d: nc.dram_tensor(f"fused_{d}", [VOCAB, 6 * 128], DT, kind="ExternalInput")
        for d in "fb"
    }
    WDT = F8 if USE_FP8_H else DT
    whh_dd = {
        d: nc.dram_tensor(f"whh_{d}", [2, 128, 8 * 128], WDT, kind="ExternalInput")
        for d in "fb"
    }
    hout_d = nc.dram_tensor("hout", [128, 4 * NW], DT, kind="ExternalOutput")

    with tile.TileContext(nc) as tc:
        with (
            tc.tile_pool(name="const", bufs=1) as cpool,
            tc.tile_pool(name="work", bufs=2) as wpool,
            tc.tile_pool(name="state", bufs=2) as spool,
            tc.tile_pool(name="psum", bufs=4, space=bass.MemorySpace.PSUM) as ppool,
        ):
            # --- constants / inputs --------------------------------------
            # t=0 slabs first (small DMAs) so step 0 can start immediately.
            xwi_sb = cpool.tile([128, 2 * L * 2 * NW], DT, name="xwi_sb", tag="xwi")
            xwi3 = xwi_sb[:].rearrange("p (s n) -> p s n", s=2 * L)
            nc.sync.dma_start(xwi3[:, 0:2], xwi_d[0:2].rearrange("s p n -> p s n"))

            fused = {}
            oh_ends = {}
            for d, te in (("f", 0), ("b", L - 1)):
                fu = cpool.tile([128, 6 * 128], DT, name=f"fused_{d}_sb", tag=f"fu_{d}")
                nc.sync.dma_start(fu[:], fused_dd[d][:])
                fused[d] = fu
                ot = cpool.tile([128, NW], DT, name=f"oh_e{te}", tag=f"oh_e{te}")
                nc.sync.dma_start(ot[:], oh_d[te])
                oh_ends[te] = ot
            whh = {}
            for d in "fb":
                w = cpool.tile([128, 2 * 8 * 128], WDT, name=f"whh_{d}_sb", tag=f"whh_{d}")
                nc.sync.dma_start(
                    w[:].rearrange("p (k g) -> p k g", k=2),
                    whh_dd[d].rearrange("k p g -> p k g"),
                )
                if USE_FP8_H:
                    # 3D view [p, k=2, 8*128] for DoubleRow lhsT slicing
                    whh[d] = w[:].rearrange("p (k g) -> p k g", k=2)
                else:
                    whh[d] = [w[:, 0 : 8 * 128], w[:, 8 * 128 : 16 * 128]]
            oh_mid = {}
            for lo, hi in ((1, 8), (8, 15)):
                om = cpool.tile([128, 7 * NW], DT, name=f"oh_m{lo}", tag=f"oh_m{lo}")
                nc.sync.dma_start(
                    om[:].rearrange("p (t n) -> p t n", t=7),
                    oh_d[lo:hi].rearrange("t p n -> p t n"),
                )
                oh_mid[lo] = om
            # rest of xwi, per-step slabs so step t only waits on its own pair
            for s in range(2, 2 * L, 2):
                nc.sync.dma_start(
                    xwi3[:, s : s + 2],
                    xwi_d[s : s + 2].rearrange("s p n -> p s n"),
                )

            def oh_rhs(t):
                if t in oh_ends:
                    return oh_ends[t][:]
                lo = 1 if t < 8 else 8
                return oh_mid[lo][:, (t - lo) * NW : (t - lo + 1) * NW]

            def xwi_slice(d, t):
                s = 2 * t + (0 if d == "f" else 1)
                return xwi_sb[:, s * 2 * NW : (s + 1) * 2 * NW]

            out_sb = cpool.tile([128, 4 * NW], DT, name="out_sb", tag="out_sb")
            zero_sb = cpool.tile([128, 2 * NW], DT, name="zero_sb", tag="zero_sb")
            nc.vector.memset(zero_sb[:], 0.0)
            # GPSIMD warm-up: the first tensor op on Pool pays a ~6us IRAM
            # kernel load; pay it here (overlapped with input DMAs) instead of
            # on the t=1 cell-update chain.
            gp_warm = cpool.tile([128, 128], DT, name="gp_warm", tag="gp_warm")
            nc.gpsimd.tensor_mul(gp_warm[:], zero_sb[:, 0:128], zero_sb[:, 0:128])
            gp_warm8 = cpool.tile([128, 128], F8, name="gp_warm8", tag="gp_warm8")
            nc.gpsimd.tensor_copy(gp_warm8[:], gp_warm[:])

            # HAM warm-up: dummy matmuls while input DMAs are in flight.
            warm_src = wpool.tile([128, NW], DT, name="warm_src", tag="warm_src", bufs=1)
            nc.gpsimd.memset(warm_src[:], 0.0)
            warm_ps = ppool.tile([128, 2 * NW], F32, name="warm_ps", tag="ps")
            for wj in range(22):
                nc.tensor.matmul(
                    warm_ps[:, (wj % 8) * 128 : (wj % 8) * 128 + 128],
                    warm_src[:, 0:128],
                    warm_src[:, 0:128],
                    start=True,
                    stop=True,
                )

            c2_cur = {"f": None, "b": None}
            h_cur = {"f": None, "b": None}

            # --- per-step-dir emission helpers ----------------------------
            def emit_mms(d, t):
                tchar = t if d == "f" else L - 1 - t
                rhs_oh = oh_rhs(tchar)
                h_prev = h_cur[d]
                first = h_prev is None
                ps = {}
                # one-hot LUT matmuls (gate i has none; fused col order f,o,g)
                for g in _OH_ORDER:
                    ps[g] = ppool.tile([128, 2 * NW], F32, name=f"p_{g}", tag="ps")
                    oc = 2 * _GSL[g]
                    for j in range(2):
                        nc.tensor.matmul(
                            ps[g][:, j * NW : (j + 1) * NW],
                            fused[d][:, (oc + j) * 128 : (oc + j + 1) * 128],
                            rhs_oh,
                            start=True,
                            stop=first,
                        )
                if not first:
                    ps["i"] = ppool.tile([128, 2 * NW], F32, name="p_i", tag="ps")
                    # recurrent matmuls; whh slice order f0 f1 o0 o1 g0 g1 i0 i1
                    if USE_FP8_H:
                        rhs_h = h_prev[:].rearrange("p (k n) -> p k n", k=2)
                        for g in _H_ORDER:
                            for j in range(2):
                                sl = 2 * _GSL[g] + j
                                nc.tensor.matmul(
                                    ps[g][:, j * NW : (j + 1) * NW],
                                    whh[d][:, :, sl * 128 : (sl + 1) * 128],
                                    rhs_h,
                                    start=(g == "i"),
                                    stop=True,
                                    perf_mode=mybir.MatmulPerfMode.DoubleRow,
                                )
                    else:
                        for g in _H_ORDER:
                            for j in range(2):
                                sl = 2 * _GSL[g] + j
                                for k in range(2):
                                    nc.tensor.matmul(
                                        ps[g][:, j * NW : (j + 1) * NW],
                                        whh[d][k][:, sl * 128 : (sl + 1) * 128],
                                        h_prev[:, k * NW : (k + 1) * NW],
                                        start=(g == "i" and k == 0),
                                        stop=(k == 1),
                                    )
                return ps

            def emit_act(d, ps):
                sf = wpool.tile([128, 2 * NW], DT, name="sf", tag=f"sf_{d}")
                nc.scalar.activation(sf[:], ps["f"][:], AFT.Sigmoid)
                so = wpool.tile([128, 2 * NW], DT, name="so", tag=f"so_{d}")
                nc.scalar.activation(so[:], ps["o"][:], AFT.Sigmoid)
                tg = wpool.tile([128, 2 * NW], DT, name="tg", tag=f"tg_{d}")
                nc.scalar.activation(tg[:], ps["g"][:], AFT.Tanh)
                return sf, so, tg

            def emit_cell(d, t, ps, sf, tg):
                # s2i = 2*sigmoid(i)
                s2i = wpool.tile([128, 2 * NW], DT, name="s2i", tag=f"s2i_{d}")
                if t == 0:
                    nc.vector._custom_dve(
                        ADDSIG2, out=s2i[:], in0=xwi_slice(d, t), in1=zero_sb[:],
                        s0=P0, s1=P1, imm2=P2,
                    )
                else:
                    nc.vector._custom_dve(
                        ADDSIG2, out=s2i[:], in0=ps["i"][:], in1=xwi_slice(d, t),
                        s0=P0, s1=P1, imm2=P2,
                    )
                # m2 = 2*sig_i*tanh_g
                m2 = wpool.tile([128, 2 * NW], DT, name="m2", tag=f"m2_{d}")
                nc.vector.tensor_mul(m2[:], s2i[:], tg[:])
                if t == 0:
                    c2_cur[d] = m2
                    return
                # m1 = sig_f * c2_prev  (GPSIMD: off the critical path)
                m1 = wpool.tile([128, 2 * NW], DT, name="m1", tag=f"m1_{d}")
                nc.gpsimd.tensor_mul(m1[:], sf[:], c2_cur[d][:])
                c2 = spool.tile([128, 2 * NW], DT, name=f"c2_{d}", tag=f"c2_{d}")
                nc.vector.tensor_add(c2[:], m1[:], m2[:])
                c2_cur[d] = c2

            def emit_h(d, t, so):
                # h = tanh(c) * sig_o = p(c2) * sig_o
                if t == L - 1:
                    off = 0 if d == "f" else 2 * NW
                    out_ap = out_sb[:, off : off + 2 * NW]
                else:
                    hdt = F8 if USE_FP8_H else DT
                    h = spool.tile([128, 2 * NW], hdt, name=f"h_{d}", tag=f"h_{d}")
                    out_ap = h[:]
                    h_cur[d] = h
                nc.vector._custom_dve(
                    TANHMUL, out=out_ap, in0=c2_cur[d][:], in1=so[:],
                    s0=P0, s1=P1, imm2=P2,
                )

            # --- the recurrence, f/b interleaved --------------------------
            for t in range(L):
                ps_f = emit_mms("f", t)
                sf_f, so_f, tg_f = emit_act("f", ps_f)
                ps_b = emit_mms("b", t)
                emit_cell("f", t, ps_f, sf_f, tg_f)
                sf_b, so_b, tg_b = emit_act("b", ps_b)
                emit_h("f", t, so_f)
                emit_cell("b", t, ps_b, sf_b, tg_b)
                emit_h("b", t, so_b)

            nc.sync.dma_start(hout_d[:, 0 : 2 * NW], out_sb[:, 0 : 2 * NW])
            nc.sync.dma_start(hout_d[:, 2 * NW : 4 * NW], out_sb[:, 2 * NW : 4 * NW])

    nc.compile()
    return nc


_NC_CACHE = None


def _get_nc():
    global _NC_CACHE
    if _NC_CACHE is None:
        _NC_CACHE = build_nc()
    return _NC_CACHE


def _np_dt(dt):
    return mybir.dt.np(dt)


def prepare_in_maps(x, embed_table, w_ih_f, w_hh_f, b_ih_f, b_hh_f,
                    w_ih_b, w_hh_b, b_ih_b, b_hh_b):
    cdt = _np_dt(DT)
    ids = np.asarray(x).reshape(B * T, L).astype(np.int64)

    shared = {}
    fused_i = {}
    for d, w_ih, w_hh, b_ih, b_hh in (
        ("f", w_ih_f, w_hh_f, b_ih_f, b_hh_f),
        ("b", w_ih_b, w_hh_b, b_ih_b, b_hh_b),
    ):
        w_ih = np.asarray(w_ih, np.float32)
        w_hh = np.asarray(w_hh, np.float32)
        b = np.asarray(b_ih, np.float32) + np.asarray(b_hh, np.float32)
        fused_full = np.asarray(embed_table, np.float32) @ w_ih.T + b[None, :]
        shared[f"fused_{d}"] = np.ascontiguousarray(
            fused_full[:, _FOG_COLS].astype(cdt)
        )
        fused_i[d] = np.ascontiguousarray(fused_full[:, 0:256].astype(cdt))
        whh_t = w_hh.T[:, _HSLICE_COLS]  # [256, 8*128] cols = slice-major
        wdt = _np_dt(F8) if USE_FP8_H else cdt
        shared[f"whh_{d}"] = np.ascontiguousarray(
            whh_t.reshape(2, 128, 8 * 128).astype(wdt)
        )

    vrange = np.arange(VOCAB)
    in_maps = []
    for c in range(N_CORES):
        ids_c = ids[c * NW : (c + 1) * NW]  # [NW, L]
        oh = (ids_c.T[:, None, :] == vrange[None, :, None]).astype(cdt)  # [L,V,NW]
        # xwi[s=2t+dir]: [128, 2*NW] with col = k*NW + word
        xwi = np.empty((2 * L, 128, 2 * NW), cdt)
        for t in range(L):
            for di, d in enumerate("fb"):
                tchar = t if d == "f" else L - 1 - t
                g = fused_i[d][ids_c[:, tchar]]  # [NW, 256]
                xwi[2 * t + di] = (
                    g.T.reshape(2, 128, NW).transpose(1, 0, 2).reshape(128, 2 * NW)
                )
        m = dict(shared)
        m["oh"] = np.ascontiguousarray(oh)
        m["xwi"] = np.ascontiguousarray(xwi)
        in_maps.append(m)
    return in_maps


def assemble_output(results):
    ys = []
    for c in range(N_CORES):
        hout = results[c]["hout"].astype(np.float32)  # [128, 4*NW]
        hf = np.concatenate([hout[:, 0:NW], hout[:, NW : 2 * NW]], axis=0)  # [H,NW]
        hb = np.concatenate([hout[:, 2 * NW : 3 * NW], hout[:, 3 * NW : 4 * NW]], axis=0)
        ys.append(np.concatenate([hf.T, hb.T], axis=1))  # [NW, 2H]
    y = np.concatenate(ys, axis=0)  # [B*T, 2H]
    return y.reshape(B, T, 2 * H)


def run(in_maps, trace=False):
    nc = _get_nc()
    res = run_bass_kernel_spmd(nc, in_maps, core_ids=list(range(N_CORES)), trace=trace)
    return res


def kernel(**inputs) -> np.ndarray:
    in_maps = prepare_in_maps(**inputs)
    res = run(in_maps, trace=False)
    return assemble_output(res.results)
